# revision 7
# baseline (speedup 1.0000x reference)
"""Trainium2 Bass kernel for the CNNFusing ragged-session attention pooling module.

Computes, per session s over its contiguous token range:
    v_mean   = mean(hidden[s])                                  [H]
    pos_h[t] = tanh(hidden[t] @ Wp1 + (pos_table @ Wp2 + b_pos)[rp[t]])
    gate[t]  = sigmoid(v_mean @ W1 + b1 + pos_h[t] @ W2 + b2)
    alpha[t] = gate[t] @ qw + qb
    h_s      = sum_t alpha[t] * hidden[t]                       [B, H]

Strategy: pure data parallelism over sessions across 8 cores. Each core's
sessions are packed into fixed 512-token chunks (sessions never straddle a
chunk). All ragged ops (segment sum, per-token broadcast of session values,
position-table gather) become one-hot matmuls on the PE array. Operands are
fp16 (fp32 PSUM accumulation); sigmoid is folded into tanh so ScalarE keeps a
single activation table.
"""

import numpy as np

import concourse.bass as bass
import concourse.mybir as mybir
import concourse.tile as tile
from concourse.vector_clock import ScopedClock
from concourse.bass_utils import run_bass_kernel_spmd

H = 256
TC = 512      # tokens per chunk
S = 32        # max sessions per chunk (observed max ~14 for this distribution)
KT = TC // 128  # 128-token k-tiles per chunk
N_CORES = 8

F16 = mybir.dt.float16
F32 = mybir.dt.float32


# --------------------------------------------------------------------------
# The walrus build here accepts only ONE sync-wait command per instruction,
# while Tile may attach several (tail drain, DMA transposes, ...). Hoist all
# but the last wait of such instructions onto standalone event-semaphore
# waits inserted just before them on the same engine (sequencer executes in
# order, so semantics are preserved).
_waitsplit_uid = [0]


def _split_multi_waits(nc):
    for fn in nc.m.functions:
        for bb in fn.blocks:
            insts = bb.instructions
            i = 0
            while i < len(insts):
                inst = insts[i]
                si = getattr(inst, "sync_info", None)
                waits = list(si.on_wait) if si is not None and si.on_wait else []
                if len(waits) > 1:
                    si.on_wait = waits[-1:]
                    for w in waits[:-1]:
                        ev = mybir.InstEventSemaphore(
                            name=f"I-waitsplit-{_waitsplit_uid[0]}", ins=[], outs=[]
                        )
                        _waitsplit_uid[0] += 1
                        ev.engine = inst.engine
                        ev.sync_info = mybir.SyncInfo(on_wait=[w], on_update=[])
                        insts.insert(i, ev)
                        i += 1
                i += 1
# --------------------------------------------------------------------------


def _plan(seq_len):
    """Assign contiguous sessions to cores (balanced tokens), then pack each
    core's sessions into chunks of <= TC tokens and <= S sessions."""
    lens = np.asarray(seq_len, dtype=np.int64)
    B = len(lens)
    cum = np.cumsum(lens)
    total = int(cum[-1])
    starts = cum - lens  # token start of each session

    bounds = [0]
    for i in range(1, N_CORES):
        bounds.append(int(np.searchsorted(cum, total * i / N_CORES)))
    bounds.append(B)

    core_chunks = []
    for c in range(N_CORES):
        lo, hi = bounds[c], bounds[c + 1]
        out = []
        s = lo
        while s < hi:
            e = s
            tok = 0
            while e < hi and e - s < S and tok + lens[e] <= TC:
                tok += int(lens[e])
                e += 1
            assert e > s, "single session longer than chunk"
            out.append((s, e))
            s = e
        core_chunks.append(out)
    C = max(len(x) for x in core_chunks)
    return lens, starts, core_chunks, C


def _pack_inputs(hidden, reverse_pos, lens, starts, core_chunks, C):
    xt16 = np.zeros((N_CORES, C, TC, H), np.float16)
    seg_row = np.full((N_CORES, C, TC), -1.0, np.float16)
    rp_row = np.zeros((N_CORES, C, TC), np.float16)
    recip = np.zeros((N_CORES, C, S), np.float32)

    out_core = np.zeros(len(lens), np.int32)
    out_chunk = np.zeros(len(lens), np.int32)
    out_local = np.zeros(len(lens), np.int32)

    hidden16 = hidden.astype(np.float16)
    rp = np.asarray(reverse_pos)
    for core, chs in enumerate(core_chunks):
        for ci, (s, e) in enumerate(chs):
            t0 = int(starts[s])
            ntok = int(lens[s:e].sum())
            ns = e - s
            xt16[core, ci, :ntok] = hidden16[t0 : t0 + ntok]
            seg_row[core, ci, :ntok] = np.repeat(
                np.arange(ns, dtype=np.float16), lens[s:e]
            )
            rp_row[core, ci, :ntok] = rp[t0 : t0 + ntok].astype(np.float16)
            recip[core, ci, :ns] = 1.0 / lens[s:e]
            out_core[s:e] = core
            out_chunk[s:e] = ci
            out_local[s:e] = np.arange(ns)

    seg_col = (
        seg_row.reshape(N_CORES, C, KT, 128).transpose(0, 1, 3, 2).astype(np.float32)
    )
    return xt16, seg_row, rp_row, seg_col, recip, (out_core, out_chunk, out_local)


def _pack_weights(pos_table, W_pos, b_pos, W1, b1, W2, b2, qw, qb):
    Wp = np.asarray(W_pos, np.float32)
    wp1 = Wp[:H]
    pwf = np.asarray(pos_table, np.float32) @ Wp[H:] + np.asarray(b_pos, np.float32)
    pw = np.zeros((H, H), np.float32)
    pw[: pwf.shape[0]] = pwf

    def pack_lhsT(M):  # [256, 256] -> [128, 2, 256] (c_in half-major)
        return (
            np.ascontiguousarray(
                M.reshape(2, 128, H).transpose(1, 0, 2)
            ).astype(np.float16)
        )

    wp1p = pack_lhsT(wp1)
    pwp = pack_lhsT(pw)
    w1p = pack_lhsT(np.asarray(W1, np.float32))
    w2p = pack_lhsT(np.asarray(W2, np.float32))

    qwf = np.asarray(qw, np.float32).reshape(H)
    # alpha = gate@qw + qb with gate = 0.5*gt + 0.5 folds to
    # alpha = 0.5*(gt@qw) + (qb + sum(qw)/2); the 0.5 is applied post-matmul.
    qwh = np.ascontiguousarray(qwf.reshape(2, 128).T).astype(np.float16)
    qbp = float(np.asarray(qb, np.float32).reshape(()) + qwf.sum() / 2.0)
    bcf = 0.5 * (np.asarray(b1, np.float32) + np.asarray(b2, np.float32))
    bch = np.ascontiguousarray(bcf.reshape(2, 128).T).astype(np.float32)

    iota_at = np.broadcast_to(
        np.arange(S, dtype=np.float16), (128, S)
    ).copy()
    iota_p = np.stack(
        [np.arange(128, dtype=np.float32), np.arange(128, 256, dtype=np.float32)], 1
    )
    iota_s = np.arange(S, dtype=np.float32).reshape(S, 1)
    return dict(
        wp1=wp1p, pw=pwp, w1=w1p, w2=w2p, qwh=qwh, bch=bch,
        iota_at=iota_at, iota_p=iota_p, iota_s=iota_s,
    ), qbp


def _build_bass(C, qbp):
    nc = bass.Bass("TRN2", target_bir_lowering=False, debug=False,
                   num_devices=N_CORES)

    xt = nc.dram_tensor("xt", [C, TC, H], F16, kind="ExternalInput")
    seg_row = nc.dram_tensor("seg_row", [C, TC], F16, kind="ExternalInput")
    rp_row = nc.dram_tensor("rp_row", [C, TC], F16, kind="ExternalInput")
    seg_col = nc.dram_tensor("seg_col", [C, 128, KT], F32, kind="ExternalInput")
    recip = nc.dram_tensor("recip", [C, S], F32, kind="ExternalInput")
    wp1 = nc.dram_tensor("wp1", [128, 2, H], F16, kind="ExternalInput")
    pw = nc.dram_tensor("pw", [128, 2, H], F16, kind="ExternalInput")
    w1 = nc.dram_tensor("w1", [128, 2, H], F16, kind="ExternalInput")
    w2 = nc.dram_tensor("w2", [128, 2, H], F16, kind="ExternalInput")
    qwh = nc.dram_tensor("qwh", [128, 2], F16, kind="ExternalInput")
    bch = nc.dram_tensor("bch", [128, 2], F32, kind="ExternalInput")
    iota_at = nc.dram_tensor("iota_at", [128, S], F16, kind="ExternalInput")
    iota_p = nc.dram_tensor("iota_p", [128, 2], F32, kind="ExternalInput")
    iota_s = nc.dram_tensor("iota_s", [S, 1], F32, kind="ExternalInput")
    hs = nc.dram_tensor("hs", [C, S, H], F32, kind="ExternalOutput")

    eq = mybir.AluOpType.is_equal
    mult = mybir.AluOpType.mult
    add = mybir.AluOpType.add
    Tanh = mybir.ActivationFunctionType.Tanh

    with tile.TileContext(nc) as tc:
        with (
            tc.tile_pool(name="consts", bufs=1) as pc,
            tc.tile_pool(name="work", bufs=3) as pw_pool,
            # PSUM is 8 banks: bcast 2 + ph 2 + gate 2 + small 1 + small2 1
            tc.tile_pool(name="pbc", bufs=2, space="PSUM") as ppb,
            tc.tile_pool(name="pbig", bufs=1, space="PSUM") as ppbig,
            tc.tile_pool(name="psmall", bufs=1, space="PSUM") as pps,
        ):
            # ---- constants ----
            wp1_sb = pc.tile([128, 2, H], F16)
            nc.sync.dma_start(out=wp1_sb, in_=wp1[:])
            pw_sb = pc.tile([128, 2, H], F16)
            nc.sync.dma_start(out=pw_sb, in_=pw[:])
            w1_sb = pc.tile([128, 2, H], F16)
            nc.sync.dma_start(out=w1_sb, in_=w1[:])
            w2_sb = pc.tile([128, 2, H], F16)
            nc.sync.dma_start(out=w2_sb, in_=w2[:])
            qwh_sb = pc.tile([128, 2], F16)
            nc.sync.dma_start(out=qwh_sb, in_=qwh[:])
            bch_sb = pc.tile([128, 2], F32)
            nc.sync.dma_start(out=bch_sb, in_=bch[:])
            iota_at_sb = pc.tile([128, S], F16)
            nc.sync.dma_start(out=iota_at_sb, in_=iota_at[:])
            iota_p_sb = pc.tile([128, 2], F32)
            nc.sync.dma_start(out=iota_p_sb, in_=iota_p[:])
            iota_s_sb = pc.tile([S, 1], F32)
            nc.sync.dma_start(out=iota_s_sb, in_=iota_s[:])
            ones_sb = pc.tile([1, 128], F16)
            nc.vector.memset(ones_sb, 1.0)

            for c in range(C):
                # ---- loads ----
                x = pw_pool.tile([128, KT, H], F16, tag="x")
                nc.sync.dma_start(
                    out=x, in_=xt[c].rearrange("(k p) h -> p k h", p=128)
                )
                xT0 = pw_pool.tile([128, TC], F16, tag="xT0")
                nc.sync.dma_start_transpose(out=xT0, in_=xt[c][:, 0:128])
                xT1 = pw_pool.tile([128, TC], F16, tag="xT1")
                nc.sync.dma_start_transpose(out=xT1, in_=xt[c][:, 128:256])
                segr = pw_pool.tile([1, TC], F16, tag="segr")
                nc.sync.dma_start(out=segr, in_=seg_row[c][None, :])
                rpr = pw_pool.tile([1, TC], F16, tag="rpr")
                nc.sync.dma_start(out=rpr, in_=rp_row[c][None, :])
                segc = pw_pool.tile([128, KT], F32, tag="segc")
                nc.sync.dma_start(out=segc, in_=seg_col[c])
                rec = pw_pool.tile([S, 1], F32, tag="rec")
                nc.sync.dma_start(out=rec, in_=recip[c][:, None])

                # ---- PE broadcasts of rp/seg along partitions ----
                rpb = ppb.tile([128, TC], F32, tag="bcast")
                nc.tensor.matmul(rpb, ones_sb[:1, :128], rpr, start=True, stop=True)
                segb = ppb.tile([S, TC], F32, tag="bcast")
                nc.tensor.matmul(segb, ones_sb[:1, :S], segr, start=True, stop=True)

                # ---- one-hot masks ----
                poh = pw_pool.tile([128, 2, TC], F16, tag="poh")
                nc.vector.tensor_single_scalar(
                    out=poh[:, 0, :], in_=rpb, scalar=iota_p_sb[:, 0:1], op=eq
                )
                nc.vector.tensor_single_scalar(
                    out=poh[:, 1, :], in_=rpb, scalar=iota_p_sb[:, 1:2], op=eq
                )
                a_s = pw_pool.tile([S, TC], F16, tag="a_s")
                nc.vector.tensor_single_scalar(
                    out=a_s, in_=segb, scalar=iota_s_sb, op=eq
                )
                a_t = pw_pool.tile([128, KT, S], F16, tag="a_t")
                for k in range(KT):
                    nc.vector.tensor_single_scalar(
                        out=a_t[:, k, :], in_=iota_at_sb,
                        scalar=segc[:, k : k + 1], op=eq,
                    )

                # ---- session sums, transposed: ss[c_half*64+s] ----
                ss = pps.tile([128, 2 * S], F32, tag="small")
                for h in range(2):
                    for k in range(KT):
                        nc.tensor.matmul(
                            ss[:, h * S : (h + 1) * S],
                            x[:, k, h * 128 : (h + 1) * 128],
                            a_t[:, k, :],
                            start=(k == 0),
                            stop=(k == KT - 1),
                        )
                smt = pw_pool.tile([128, 2 * S], F16, tag="smt")
                nc.vector.tensor_copy(out=smt, in_=ss)

                # ---- G1 = (sess_mean @ W1), scaled by 1/len ----
                g1p = pps.tile([S, H], F32, tag="small2")
                for k in range(2):
                    nc.tensor.matmul(
                        g1p, smt[:, k * S : (k + 1) * S], w1_sb[:, k, :],
                        start=(k == 0), stop=(k == 1),
                    )
                g1 = pw_pool.tile([S, H], F16, tag="g1")
                nc.vector.tensor_single_scalar(
                    out=g1, in_=g1p, scalar=rec, op=mult
                )

                # ---- pos_hidden = tanh(Wp1 @ x + PW @ onehot) ----
                php = ppbig.tile([128, 2 * TC], F32, tag="ph")
                xTs = (xT0, xT1)
                for h in range(2):
                    dst = php[:, h * TC : (h + 1) * TC]
                    hs_lo, hs_hi = h * 128, (h + 1) * 128
                    nc.tensor.matmul(dst, wp1_sb[:, 0, hs_lo:hs_hi], xTs[0],
                                     start=True, stop=False)
                    nc.tensor.matmul(dst, wp1_sb[:, 1, hs_lo:hs_hi], xTs[1],
                                     start=False, stop=False)
                    nc.tensor.matmul(dst, pw_sb[:, 0, hs_lo:hs_hi], poh[:, 0, :],
                                     start=False, stop=False)
                    nc.tensor.matmul(dst, pw_sb[:, 1, hs_lo:hs_hi], poh[:, 1, :],
                                     start=False, stop=True)
                ph = pw_pool.tile([128, 2 * TC], F16, tag="ph_sb")
                for h in range(2):
                    nc.scalar.activation(
                        out=ph[:, h * TC : (h + 1) * TC],
                        in_=php[:, h * TC : (h + 1) * TC],
                        func=Tanh,
                    )

                # ---- gate: sigmoid(V + W2 @ ph + bc) via tanh ----
                gp = ppbig.tile([128, 2 * TC], F32, tag="gate")
                for h in range(2):
                    dst = gp[:, h * TC : (h + 1) * TC]
                    hs_lo, hs_hi = h * 128, (h + 1) * 128
                    nc.tensor.matmul(dst, g1[:, hs_lo:hs_hi], a_s,
                                     start=True, stop=False)
                    nc.tensor.matmul(dst, w2_sb[:, 0, hs_lo:hs_hi], ph[:, 0:TC],
                                     start=False, stop=False)
                    nc.tensor.matmul(dst, w2_sb[:, 1, hs_lo:hs_hi], ph[:, TC:],
                                     start=False, stop=True)
                gt = pw_pool.tile([128, 2 * TC], F16, tag="gt")
                for h in range(2):
                    nc.scalar.activation(
                        out=gt[:, h * TC : (h + 1) * TC],
                        in_=gp[:, h * TC : (h + 1) * TC],
                        func=Tanh, scale=0.5, bias=bch_sb[:, h : h + 1],
                    )

                # ---- alpha[t] = gate @ qw + qb (gate=0.5*gt+0.5 folded) ----
                alp = pps.tile([128, KT], F32, tag="small")
                for kt in range(KT):
                    for h in range(2):
                        nc.tensor.matmul(
                            alp[:, kt : kt + 1],
                            gt[:, h * TC + kt * 128 : h * TC + (kt + 1) * 128],
                            qwh_sb[:, h : h + 1],
                            start=(h == 0), stop=(h == 1),
                        )
                alpha = pw_pool.tile([128, KT], F32, tag="alpha")
                nc.vector.tensor_scalar(
                    out=alpha, in0=alp, scalar1=0.5, scalar2=qbp,
                    op0=mult, op1=add,
                )
                aat = pw_pool.tile([128, KT, S], F16, tag="aat")
                for k in range(KT):
                    nc.vector.tensor_single_scalar(
                        out=aat[:, k, :], in_=a_t[:, k, :],
                        scalar=alpha[:, k : k + 1], op=mult,
                    )

                # ---- h_s = A_alpha.T @ x ----
                hsp = pps.tile([S, H], F32, tag="small2")
                for k in range(KT):
                    nc.tensor.matmul(
                        hsp, aat[:, k, :], x[:, k, :],
                        start=(k == 0), stop=(k == KT - 1),
                    )
                hs_sb = pw_pool.tile([S, H], F32, tag="hs_sb")
                nc.vector.tensor_copy(out=hs_sb, in_=hsp)
                nc.sync.dma_start(out=hs[c], in_=hs_sb)

    _split_multi_waits(nc)
    return nc


_CACHE = {}


def kernel(hidden, pos_table, W_pos, b_pos, W1, b1, W2, b2, qw, qb,
           seq_len, reverse_pos):
    hidden = np.asarray(hidden, np.float32)
    seq_len_np = np.asarray(seq_len)
    lens, starts, core_chunks, C = _plan(seq_len_np)
    xt16, seg_row, rp_row, seg_col, recip, unpack_idx = _pack_inputs(
        hidden, reverse_pos, lens, starts, core_chunks, C
    )
    weights, qbp = _pack_weights(
        pos_table, W_pos, b_pos, W1, b1, W2, b2, qw, qb
    )

    key = (C, qbp)
    if key not in _CACHE:
        _CACHE[key] = _build_bass(C, qbp)
    nc = _CACHE[key]

    in_maps = []
    for core in range(N_CORES):
        m = dict(
            xt=xt16[core], seg_row=seg_row[core], rp_row=rp_row[core],
            seg_col=seg_col[core], recip=recip[core],
        )
        m.update(weights)
        in_maps.append(m)

    import time as _time

    t0 = _time.perf_counter()
    res = run_bass_kernel_spmd(nc, in_maps, core_ids=list(range(N_CORES)))
    kernel._last_run_s = _time.perf_counter() - t0
    hs_all = np.stack([res.results[i]["hs"] for i in range(N_CORES)])

    out_core, out_chunk, out_local = unpack_idx
    return np.ascontiguousarray(hs_all[out_core, out_chunk, out_local])
